# revision 27
# baseline (speedup 1.0000x reference)
"""Trainium2 Bass kernel for nn_MHInrAttn (sparse_attention, b=4 s=1024 f=1024 h=16).

v2 sharding (8 NeuronCores): core c -> (batch c//2, head-group c%2).
Each core: 8 heads of one batch. The reference uses a raw .reshape with NO
transpose, so head h's Q/K/V come from rows [64h, 64h+64) of the projected
[s, f] matrix -> a head-group needs only x rows [512g, 512g+512).

Per core: project Q/K/V for 512 rows, attention for 8 heads in scores^T
[k, q] orientation, partial output projection through its 512 Wo rows.
Host: sum partial pairs per batch, transpose, add bo.

Device details:
  - all matmul operands bf16 (1 cyc/row on PE), fp32 PSUM accumulation
  - str^T streams causally trimmed into a PACKED eM tile (one wide exp/head)
  - softmax-1 k-sums via ones-column PE matmul; 1/x via Ln->Exp on ACT with
    Ln/Exp batched per phase (they live in different ACT table sets)
  - qk pair tile [128, 1024] spans 2 PSUM banks -> wide ACT/DVE ops
  - PV carries a ones column in V for softmax-2 row sums for free
"""

import numpy as np

B, S, F, H, D = 4, 1024, 1024, 16, 64
NCORES = 8
HG = 8           # heads per core (head-group)
RW = 512         # proj rows per core
P = 128
NEG_FILL = -40.0

_CACHE = {}

_WOFF = [0, 1024, 1920, 2688, 3328, 3840, 4224, 4480]  # packed eM offsets
_WTOT = 4608


def _np_bf16():
    import ml_dtypes

    return ml_dtypes.bfloat16


def _build_nc(causal=True):
    from contextlib import ExitStack

    import concourse.bacc as bacc
    import concourse.tile as tile
    from concourse import mybir

    dt = mybir.dt
    f32 = dt.float32
    bf16 = dt.bfloat16
    Exp = mybir.ActivationFunctionType.Exp
    Ln = mybir.ActivationFunctionType.Ln

    nc = bacc.Bacc("TRN2", target_bir_lowering=False, debug=False)

    xT_d = nc.dram_tensor("xT", [F, RW], bf16, kind="ExternalInput").ap()
    str_d = nc.dram_tensor("strT", [HG, S, S], bf16, kind="ExternalInput").ap()
    wq_d = nc.dram_tensor("wq", [F, F], bf16, kind="ExternalInput").ap()
    wk_d = nc.dram_tensor("wk", [F, F], bf16, kind="ExternalInput").ap()
    wv_d = nc.dram_tensor("wv", [F, F], bf16, kind="ExternalInput").ap()
    wo_d = nc.dram_tensor("wo", [RW, F], bf16, kind="ExternalInput").ap()
    bias_d = nc.dram_tensor("bqkv", [3, F], bf16, kind="ExternalInput").ap()
    ident_d = nc.dram_tensor("ident", [P, P], bf16, kind="ExternalInput").ap()
    out_d = nc.dram_tensor("outT", [F, S], bf16, kind="ExternalOutput").ap()

    woff = _WOFF if causal else [1024 * j for j in range(8)]
    wtot = _WTOT if causal else 8 * 1024

    with ExitStack() as ctx:
        tc = ctx.enter_context(tile.TileContext(nc))
        consts = ctx.enter_context(tc.tile_pool(name="consts", bufs=1))
        qtkt = ctx.enter_context(tc.tile_pool(name="qtkt", bufs=1))
        v2p = ctx.enter_context(tc.tile_pool(name="v2", bufs=1))
        outp = ctx.enter_context(tc.tile_pool(name="outp", bufs=1))
        wop = ctx.enter_context(tc.tile_pool(name="wop", bufs=1))
        dramp = ctx.enter_context(tc.tile_pool(name="dram", bufs=1, space="DRAM"))

        ident = consts.tile([P, P], bf16, tag="ident", name="ident")
        nc.sync.dma_start(out=ident, in_=ident_d)
        ones_all = consts.tile([P, P], bf16, tag="ones", name="ones")
        nc.vector.memset(ones_all, 1.0)
        bias_sb = consts.tile([1, 3 * F], bf16, tag="bias", name="bias")
        nc.sync.dma_start(out=bias_sb, in_=bias_d.rearrange("a b -> (a b)").unsqueeze(0))
        wo_sb = wop.tile([P, 4, F], bf16, tag="wo", name="wo")
        nc.sync.dma_start(out=wo_sb, in_=wo_d.rearrange("(c p) f -> p c f", p=P))

        QT, KT, OT, V2 = {}, {}, {}, {}
        for pr in range(4):  # head pairs
            QT[pr] = qtkt.tile([P, S], bf16, tag=f"qt{pr}", name=f"qt{pr}")
            KT[pr] = qtkt.tile([P, S], bf16, tag=f"kt{pr}", name=f"kt{pr}")
            OT[pr] = outp.tile([P, S], bf16, tag=f"ot{pr}", name=f"ot{pr}")
        for hp in range(HG):
            V2[hp] = v2p.tile([P, 8, P], bf16, tag=f"v{hp}", name=f"v{hp}")

        # ---------- phase 2 setup: str loads + exps for round 0 ----------
        # emitted before phase 1 in the ACT/DMA streams: the scalar engine is
        # otherwise idle during the projections, so round 0's eM exps are done
        # by the time the PE finishes the transposes
        emp = ctx.enter_context(tc.tile_pool(name="em", bufs=1))
        oap = ctx.enter_context(tc.tile_pool(name="oA", bufs=1))
        oA = {}
        rbcp = ctx.enter_context(tc.tile_pool(name="rbc", bufs=1))
        miscp = ctx.enter_context(tc.tile_pool(name="m", bufs=1))
        eM, r1bc, r1raw, r1sb = {}, {}, {}, {}

        def em_load(hp):
            t = emp.tile([P, wtot], bf16, tag=f"e{hp % 4}", name=f"e{hp % 4}")
            eM[hp] = t
            for j in range(8):
                jl = 128 * j if causal else 0
                w = S - jl
                nc.sync.dma_start(out=t[:, woff[j]:woff[j] + w],
                                  in_=str_d[hp, 128 * j:128 * (j + 1), jl:])
            nc.scalar.activation(t, t, Exp)

        # round 0's eM loads are emitted just after the xt/wq DMAs below (str
        # must not queue ahead of the DMAs the projections wait on); round 1's
        # are prefetched inside round 0's attention loop.

        def softmax1_sums(hps, r1psp):
            # softmax-1 denominators via ones-column matmuls
            for hp in hps:
                ps_r1 = r1psp.tile([1, S], f32, tag="r1", name="r1")
                for j in range(8):
                    jl = 128 * j if causal else 0
                    for h2 in range(2):
                        lo = max(512 * h2, jl)
                        hi = 512 * (h2 + 1)
                        if lo < hi:
                            last_j = (3 if h2 == 0 else 7) if causal else 7
                            nc.tensor.matmul(
                                ps_r1[0:1, lo:hi],
                                ones_all[:, 0:1],
                                eM[hp][:, woff[j] + lo - jl:woff[j] + hi - jl],
                                start=(j == 0), stop=(j == last_j))
                rr = miscp.tile([1, S], f32, tag=f"rr{hp % 4}", name=f"rr{hp % 4}")
                r1raw[hp] = rr
                nc.vector.tensor_copy(rr, ps_r1)

        def softmax1_finish(hps):
            # reciprocals: the round's first head goes solo (its 1/R1 gates
            # the first attention head's E-path), the rest batch their
            # Lns/Exps to limit ACT table reloads; the idle GPSIMD engine
            # broadcasts (r1sb is a partition-0 [1, S] tile)
            def recip_block(block):
                for hp in block:
                    nc.scalar.activation(r1raw[hp], r1raw[hp], Ln)
                for hp in block:
                    rs = miscp.tile([1, S], bf16, tag=f"rs{hp % 4}", name=f"rs{hp % 4}")
                    r1sb[hp] = rs
                    nc.scalar.activation(rs, r1raw[hp], Exp, scale=-1.0)
            recip_block(hps[:1])
            recip_block(hps[1:])
            for hp in hps:
                rbc = rbcp.tile([P, S], bf16, tag=f"rb{hp % 4}", name=f"rb{hp % 4}")
                r1bc[hp] = rbc
                nc.gpsimd.partition_broadcast(rbc, r1sb[hp])


        # ---------- phase 1: projections + layout shuffles ----------
        with tc.tile_pool(name="xt", bufs=1) as xtp, \
                tc.tile_pool(name="wpool", bufs=1) as wp, \
                tc.tile_pool(name="qkvc", bufs=1) as qkvcp, \
                tc.tile_pool(name="pj", bufs=2, space="PSUM") as ppool, \
                tc.tile_pool(name="tp", bufs=2, space="PSUM") as tpool:
            xt = xtp.tile([P, 8, 4, P], bf16, tag="xt", name="xt")
            nc.sync.dma_start(
                out=xt, in_=xT_d.rearrange("(kc p) (rb r) -> p kc rb r", p=P, rb=4))

            qkvc = {}
            for t_i, w_d in enumerate([wq_d, wk_d, wv_d]):
                wt = []
                for i in range(8):
                    w_tile = wp.tile([P, F], bf16, tag=f"w{i}", name=f"w{i}")
                    nc.sync.dma_start(out=w_tile, in_=w_d[i * P:(i + 1) * P, :])
                    wt.append(w_tile)
                if t_i == 0:
                    for hp in range(4):
                        em_load(hp)
                cc = qkvcp.tile([P, 4, F], bf16, tag=f"c{t_i}", name=f"c{t_i}")
                qkvc[t_i] = cc
                for rb in range(4):
                    for h2 in range(2):
                        ps = ppool.tile([P, 512], f32, tag="pj", name="pj")
                        for kc in range(8):
                            nc.tensor.matmul(
                                ps, xt[:, kc, rb, :],
                                wt[kc][:, 512 * h2:512 * (h2 + 1)],
                                start=(kc == 0), stop=False)
                        nc.tensor.matmul(
                            ps, ones_all[0:1, :],
                            bias_sb[0:1, 1024 * t_i + 512 * h2:1024 * t_i + 512 * h2 + 512],
                            start=False, stop=True)
                        nc.vector.tensor_copy(cc[:, rb, 512 * h2:512 * (h2 + 1)], ps)

            # V shuffle through DRAM into [k'-chunk partitions, d] layout (+ones)
            vs = dramp.tile([RW, F], bf16, tag="vs", name="vs")
            nc.sync.dma_start(out=vs.rearrange("(rb p) c -> p rb c", p=P),
                              in_=qkvc[2])
            for hp in range(HG):
                e = hp % 2
                nc.vector.memset(V2[hp], 0.0)
                dcol = 64 * e
                ones_col = 64 * (1 - e)
                src = vs[64 * hp:64 * hp + 64, :].rearrange(
                    "(j r) (cb d) -> (r cb) j d", j=8, cb=16)
                nc.sync.dma_start(out=V2[hp][:, :, dcol:dcol + 64], in_=src)
                nc.vector.memset(V2[hp][:, :, ones_col:ones_col + 1], 1.0)

            # round 0 softmax-1: sums right after proj (PE stays warm),
            # recips/broadcasts flow on ACT/DVE/GPSIMD under the transposes
            with tc.tile_pool(name="r1ps0", bufs=2, space="PSUM") as r1psp0:
                softmax1_sums([0, 1, 2, 3], r1psp0)
            softmax1_finish([0, 1, 2, 3])

            # Q^T / K^T via 64x64 PE transposes
            for t_i, dstmap in ((0, QT), (1, KT)):
                for hp in range(HG):
                    pr, e = hp // 2, hp % 2
                    base = 64 * e
                    src_rows = qkvc[t_i][base:base + 64, pr, :]
                    for half in range(2):
                        pst = tpool.tile([64, 512], bf16, tag="tp", name="tp")
                        for cb8 in range(8):
                            cb = 8 * half + cb8
                            nc.tensor.transpose(
                                pst[0:64, 64 * cb8:64 * cb8 + 64],
                                src_rows.rearrange("p (cb d) -> p cb d", cb=16)[:, cb, :],
                                ident[base:base + 64, base:base + 64])
                        dst = dstmap[pr][base:base + 64, :].rearrange(
                            "p (r cb) -> p r cb", cb=16)[:, :, 8 * half:8 * half + 8]
                        nc.vector.tensor_copy(
                            dst, pst[0:64, :].rearrange("p (cb8 r) -> p r cb8", cb8=8))

        # ---------- phase 2: attention, two rounds of 4 heads ----------
        for rnd in range(2):
            hps = list(range(4 * rnd, 4 * rnd + 4))
            if rnd == 1:
                with tc.tile_pool(name="r1ps1", bufs=2, space="PSUM") as r1psp:
                    softmax1_sums(hps, r1psp)
                softmax1_finish(hps)
                # round-0 half of the output projection: fills the PE while
                # the round-1 reciprocal chain runs on ACT/DVE/GPSIMD, and
                # halves the serial outproj tail after round 1
                with tc.tile_pool(name="opsA", bufs=2, space="PSUM") as opsA:
                    for fo in range(8):
                        oa = oap.tile([P, S], bf16, tag=f"oa{fo}", name=f"oa{fo}")
                        oA[fo] = oa
                        for h2 in range(2):
                            ps = opsA.tile([P, 512], f32, tag="opA", name="opA")
                            for pr in range(2):
                                nc.tensor.matmul(
                                    ps, wo_sb[:, pr, 128 * fo:128 * (fo + 1)],
                                    OT[pr][:, 512 * h2:512 * (h2 + 1)],
                                    start=(pr == 0), stop=(pr == 1))
                            nc.vector.tensor_copy(
                                oa[:, 512 * h2:512 * (h2 + 1)], ps)
            # 2d/2e: attention + PV + softmax-2 normalize
            with tc.tile_pool(name=f"ep{rnd}", bufs=3) as epool, \
                    tc.tile_pool(name=f"qk{rnd}", bufs=2, space="PSUM") as qkps, \
                    tc.tile_pool(name=f"pv{rnd}", bufs=2, space="PSUM") as pvps:
                def attention(hp, pv):
                    pr, e = hp // 2, hp % 2
                    base = 64 * e
                    for j in range(8):
                        jl = 128 * j if causal else 0
                        Ej = epool.tile([P, S], bf16, tag="E", name="E")
                        qk = qkps.tile([P, S], f32, tag="qk", name="qk")
                        for h2 in range(2):
                            nc.tensor.matmul(
                                qk[:, 512 * h2:512 * (h2 + 1)],
                                KT[pr][base:base + 64, 128 * j:128 * (j + 1)],
                                QT[pr][base:base + 64, 512 * h2:512 * (h2 + 1)],
                                start=True, stop=True)
                        # add sm into the qk PSUM so ONE full-width exp
                        # covers both the masked and live regions
                        if jl < S:
                            nc.vector.tensor_mul(
                                Ej[:, jl:], eM[hp][:, woff[j]:woff[j] + S - jl],
                                r1bc[hp][:, jl:])
                            nc.vector.tensor_add(qk[:, jl:], qk[:, jl:], Ej[:, jl:])
                        nc.scalar.activation(Ej, qk, Exp)
                        for h2 in range(2):
                            nc.tensor.matmul(
                                pv[:, 512 * h2:512 * (h2 + 1)],
                                V2[hp][:, j, :],
                                Ej[:, 512 * h2:512 * (h2 + 1)],
                                start=(j == 0), stop=(j == 7))
                # normalize rows of PV by 1/rowsum2 (ones column) in head
                # PAIRS: Ln,Ln then Exp,Exp keeps the ACT table reloads to 2
                # per pair and leaves the hp -> hp+1 boundary load-free, so
                # the next head's E-exps (and the PE behind them) never stall
                # on a table switch. DVE stages the sum row to partition 0,
                # GPSIMD broadcasts the reciprocal.
                def normalize_pair(hpA, pvA, hpB, pvB):
                    tiles = {}
                    for hp, pv in ((hpA, pvA), (hpB, pvB)):
                        e = hp % 2
                        sum_row = 64 * (1 - e)
                        r2f = miscp.tile([1, S], f32, tag=f"r2f{e}", name=f"r2f{e}")
                        r2s = miscp.tile([1, S], bf16, tag=f"r2s{e}", name=f"r2s{e}")
                        r2b = miscp.tile([P, S], bf16, tag=f"r2b{e}", name=f"r2b{e}")
                        tiles[hp] = (pv, r2f, r2s, r2b)
                        nc.vector.tensor_copy(r2f, pv[sum_row:sum_row + 1, :])
                    for hp in (hpA, hpB):
                        pv, r2f, r2s, r2b = tiles[hp]
                        nc.scalar.activation(r2f, r2f, Ln)
                    for hp in (hpA, hpB):
                        pv, r2f, r2s, r2b = tiles[hp]
                        nc.scalar.activation(r2s, r2f, Exp, scale=-1.0)
                    for hp in (hpA, hpB):
                        pv, r2f, r2s, r2b = tiles[hp]
                        pr, e = hp // 2, hp % 2
                        dlo = 64 * e
                        nc.gpsimd.partition_broadcast(r2b, r2s)
                        nc.vector.tensor_mul(OT[pr][dlo:dlo + 64, :],
                                             pv[dlo:dlo + 64, :],
                                             r2b[dlo:dlo + 64, :])

                for pi in range(2):
                    hpA, hpB = hps[2 * pi], hps[2 * pi + 1]
                    pvA = pvps.tile([P, S], f32, tag="pv", name="pv")
                    attention(hpA, pvA)
                    pvB = pvps.tile([P, S], f32, tag="pv", name="pv")
                    attention(hpB, pvB)
                    normalize_pair(hpA, pvA, hpB, pvB)
                    if rnd == 0:
                        em_load(hpA + 4)
                        em_load(hpB + 4)

        # ---------- phase 3: partial output projection ----------
        with tc.tile_pool(name="os", bufs=3) as osp, \
                tc.tile_pool(name="ops", bufs=2, space="PSUM") as opsum:
            for fo in range(8):
                ot = osp.tile([P, S], bf16, tag="os", name="os")
                for h2 in range(2):
                    ps = opsum.tile([P, 512], f32, tag="op", name="op")
                    for pr in (2, 3):
                        nc.tensor.matmul(ps, wo_sb[:, pr, 128 * fo:128 * (fo + 1)],
                                         OT[pr][:, 512 * h2:512 * (h2 + 1)],
                                         start=(pr == 2), stop=(pr == 3))
                    nc.vector.tensor_add(ot[:, 512 * h2:512 * (h2 + 1)], ps,
                                         oA[fo][:, 512 * h2:512 * (h2 + 1)])
                nc.sync.dma_start(out=out_d[128 * fo:128 * (fo + 1), :], in_=ot)

    nc.compile()
    return nc


def _prep_host(x, str_mat, attn_mask, Wq, bq, Wk, bk, Wv, bv, Wo, bo):
    bf = _np_bf16()
    x = np.asarray(x, np.float32)
    str_mat = np.asarray(str_mat, np.float32)
    attn_mask = np.asarray(attn_mask, np.float32)
    mask = attn_mask[:, 0]  # [b, s, s]
    causal = bool((mask == np.tril(np.ones((S, S), np.float32))[None]).all())
    strT = np.where(mask[:, None] == 0.0, NEG_FILL, str_mat).transpose(0, 1, 3, 2)
    strT = strT.astype(bf)
    xT = x.transpose(0, 2, 1).astype(bf)  # [b, f, s]
    Wq_s = (np.asarray(Wq, np.float32) / D).astype(bf)
    bq_s = (np.asarray(bq, np.float32) / D)
    bias = np.stack([bq_s, np.asarray(bk, np.float32),
                     np.asarray(bv, np.float32)]).astype(bf)
    Wk_c = np.asarray(Wk, np.float32).astype(bf)
    Wv_c = np.asarray(Wv, np.float32).astype(bf)
    Wo_c = np.asarray(Wo, np.float32).astype(bf)
    ident = np.eye(P, dtype=np.float32).astype(bf)
    in_maps = []
    for c in range(NCORES):
        bc, g = c // 2, c % 2
        in_maps.append({
            "xT": np.ascontiguousarray(xT[bc, :, RW * g:RW * (g + 1)]),
            "strT": np.ascontiguousarray(strT[bc, HG * g:HG * (g + 1)]),
            "wq": Wq_s, "wk": Wk_c, "wv": Wv_c,
            "wo": np.ascontiguousarray(Wo_c[RW * g:RW * (g + 1)]),
            "bqkv": bias, "ident": ident,
        })
    return in_maps, causal


def kernel(**inputs):
    from concourse.bass_utils import run_bass_kernel_spmd

    in_maps, causal = _prep_host(**inputs)
    key = ("v2", causal)
    if key not in _CACHE:
        _CACHE[key] = _build_nc(causal=causal)
    nc = _CACHE[key]
    res = run_bass_kernel_spmd(nc, in_maps, core_ids=list(range(NCORES)))
    partials = [np.asarray(r["outT"], np.float32) for r in res.results]
    out = np.stack([partials[2 * bc] + partials[2 * bc + 1] for bc in range(B)])
    out = out.transpose(0, 2, 1) + np.asarray(bo_arr(inputs), np.float32)
    return np.ascontiguousarray(out.astype(np.float32))


def bo_arr(inputs):
    return np.asarray(inputs["bo"], np.float32)


# revision 28
# speedup vs baseline: 1.0239x; 1.0239x over previous
"""Trainium2 Bass kernel for nn_MHInrAttn (sparse_attention, b=4 s=1024 f=1024 h=16).

v2 sharding (8 NeuronCores): core c -> (batch c//2, head-group c%2).
Each core: 8 heads of one batch. The reference uses a raw .reshape with NO
transpose, so head h's Q/K/V come from rows [64h, 64h+64) of the projected
[s, f] matrix -> a head-group needs only x rows [512g, 512g+512).

Per core: project Q/K/V for 512 rows, attention for 8 heads in scores^T
[k, q] orientation, partial output projection through its 512 Wo rows.
Host: sum partial pairs per batch, transpose, add bo.

Device details:
  - all matmul operands bf16 (1 cyc/row on PE), fp32 PSUM accumulation
  - str^T streams causally trimmed into a PACKED eM tile (one wide exp/head)
  - softmax-1 k-sums via ones-column PE matmul; 1/x via Ln->Exp on ACT with
    Ln/Exp batched per phase (they live in different ACT table sets)
  - qk pair tile [128, 1024] spans 2 PSUM banks -> wide ACT/DVE ops
  - PV carries a ones column in V for softmax-2 row sums for free
"""

import numpy as np

B, S, F, H, D = 4, 1024, 1024, 16, 64
NCORES = 8
HG = 8           # heads per core (head-group)
RW = 512         # proj rows per core
P = 128
NEG_FILL = -40.0

_CACHE = {}

_WOFF = [0, 1024, 1920, 2688, 3328, 3840, 4224, 4480]  # packed eM offsets
_WTOT = 4608


def _np_bf16():
    import ml_dtypes

    return ml_dtypes.bfloat16


def _build_nc(causal=True):
    from contextlib import ExitStack

    import concourse.bacc as bacc
    import concourse.tile as tile
    from concourse import mybir

    dt = mybir.dt
    f32 = dt.float32
    bf16 = dt.bfloat16
    Exp = mybir.ActivationFunctionType.Exp
    Ln = mybir.ActivationFunctionType.Ln

    nc = bacc.Bacc("TRN2", target_bir_lowering=False, debug=False)

    xT_d = nc.dram_tensor("xT", [F, RW], bf16, kind="ExternalInput").ap()
    str_d = nc.dram_tensor("strT", [HG, S, S], bf16, kind="ExternalInput").ap()
    wq_d = nc.dram_tensor("wq", [F, F], bf16, kind="ExternalInput").ap()
    wk_d = nc.dram_tensor("wk", [F, F], bf16, kind="ExternalInput").ap()
    wv_d = nc.dram_tensor("wv", [F, F], bf16, kind="ExternalInput").ap()
    wo_d = nc.dram_tensor("wo", [RW, F], bf16, kind="ExternalInput").ap()
    bias_d = nc.dram_tensor("bqkv", [3, F], bf16, kind="ExternalInput").ap()
    ident_d = nc.dram_tensor("ident", [P, P], bf16, kind="ExternalInput").ap()
    out_d = nc.dram_tensor("outT", [F, S], bf16, kind="ExternalOutput").ap()

    woff = _WOFF if causal else [1024 * j for j in range(8)]
    wtot = _WTOT if causal else 8 * 1024

    with ExitStack() as ctx:
        tc = ctx.enter_context(tile.TileContext(nc))
        consts = ctx.enter_context(tc.tile_pool(name="consts", bufs=1))
        qtkt = ctx.enter_context(tc.tile_pool(name="qtkt", bufs=1))
        v2p = ctx.enter_context(tc.tile_pool(name="v2", bufs=1))
        outp = ctx.enter_context(tc.tile_pool(name="outp", bufs=1))
        wop = ctx.enter_context(tc.tile_pool(name="wop", bufs=1))
        dramp = ctx.enter_context(tc.tile_pool(name="dram", bufs=1, space="DRAM"))

        ident = consts.tile([P, P], bf16, tag="ident", name="ident")
        nc.sync.dma_start(out=ident, in_=ident_d)
        ones_all = consts.tile([P, P], bf16, tag="ones", name="ones")
        nc.vector.memset(ones_all, 1.0)
        bias_sb = consts.tile([1, 3 * F], bf16, tag="bias", name="bias")
        nc.sync.dma_start(out=bias_sb, in_=bias_d.rearrange("a b -> (a b)").unsqueeze(0))
        wo_sb = wop.tile([P, 4, F], bf16, tag="wo", name="wo")
        nc.sync.dma_start(out=wo_sb, in_=wo_d.rearrange("(c p) f -> p c f", p=P))

        QT, KT, OT, V2 = {}, {}, {}, {}
        for pr in range(4):  # head pairs
            QT[pr] = qtkt.tile([P, S], bf16, tag=f"qt{pr}", name=f"qt{pr}")
            KT[pr] = qtkt.tile([P, S], bf16, tag=f"kt{pr}", name=f"kt{pr}")
            OT[pr] = outp.tile([P, S], bf16, tag=f"ot{pr}", name=f"ot{pr}")
        for hp in range(HG):
            V2[hp] = v2p.tile([P, 8, P], bf16, tag=f"v{hp}", name=f"v{hp}")

        # ---------- phase 2 setup: str loads + exps for round 0 ----------
        # emitted before phase 1 in the ACT/DMA streams: the scalar engine is
        # otherwise idle during the projections, so round 0's eM exps are done
        # by the time the PE finishes the transposes
        emp = ctx.enter_context(tc.tile_pool(name="em", bufs=1))
        oap = ctx.enter_context(tc.tile_pool(name="oA", bufs=1))
        oA = {}
        rbcp = ctx.enter_context(tc.tile_pool(name="rbc", bufs=1))
        miscp = ctx.enter_context(tc.tile_pool(name="m", bufs=1))
        eM, r1bc, r1raw, r1sb = {}, {}, {}, {}

        def em_load(hp):
            t = emp.tile([P, wtot], bf16, tag=f"e{hp % 4}", name=f"e{hp % 4}")
            eM[hp] = t
            for j in range(8):
                jl = 128 * j if causal else 0
                w = S - jl
                nc.sync.dma_start(out=t[:, woff[j]:woff[j] + w],
                                  in_=str_d[hp, 128 * j:128 * (j + 1), jl:])
            nc.scalar.activation(t, t, Exp)

        # round 0's eM loads are emitted just after the xt/wq DMAs below (str
        # must not queue ahead of the DMAs the projections wait on); round 1's
        # are prefetched inside round 0's attention loop.

        def softmax1_sums(hps, r1psp):
            # softmax-1 denominators via ones-column matmuls
            for hp in hps:
                ps_r1 = r1psp.tile([1, S], f32, tag="r1", name="r1")
                for j in range(8):
                    jl = 128 * j if causal else 0
                    for h2 in range(2):
                        lo = max(512 * h2, jl)
                        hi = 512 * (h2 + 1)
                        if lo < hi:
                            last_j = (3 if h2 == 0 else 7) if causal else 7
                            nc.tensor.matmul(
                                ps_r1[0:1, lo:hi],
                                ones_all[:, 0:1],
                                eM[hp][:, woff[j] + lo - jl:woff[j] + hi - jl],
                                start=(j == 0), stop=(j == last_j))
                rr = miscp.tile([1, S], f32, tag=f"rr{hp % 4}", name=f"rr{hp % 4}")
                r1raw[hp] = rr
                nc.vector.tensor_copy(rr, ps_r1)

        def softmax1_finish(hps):
            # reciprocals: the round's first head goes solo (its 1/R1 gates
            # the first attention head's E-path), the rest batch their
            # Lns/Exps to limit ACT table reloads; the idle GPSIMD engine
            # broadcasts (r1sb is a partition-0 [1, S] tile)
            def recip_block(block):
                for hp in block:
                    nc.scalar.activation(r1raw[hp], r1raw[hp], Ln)
                for hp in block:
                    rs = miscp.tile([1, S], bf16, tag=f"rs{hp % 4}", name=f"rs{hp % 4}")
                    r1sb[hp] = rs
                    nc.scalar.activation(rs, r1raw[hp], Exp, scale=-1.0)
            recip_block(hps[:1])
            recip_block(hps[1:])
            for hp in hps:
                rbc = rbcp.tile([P, S], bf16, tag=f"rb{hp % 4}", name=f"rb{hp % 4}")
                r1bc[hp] = rbc
                nc.gpsimd.partition_broadcast(rbc, r1sb[hp])


        # ---------- phase 1: projections + layout shuffles ----------
        with tc.tile_pool(name="xt", bufs=1) as xtp, \
                tc.tile_pool(name="wpool", bufs=1) as wp, \
                tc.tile_pool(name="qkvc", bufs=1) as qkvcp, \
                tc.tile_pool(name="pj", bufs=2, space="PSUM") as ppool, \
                tc.tile_pool(name="tp", bufs=2, space="PSUM") as tpool:
            xt = xtp.tile([P, 8, 4, P], bf16, tag="xt", name="xt")
            nc.sync.dma_start(
                out=xt, in_=xT_d.rearrange("(kc p) (rb r) -> p kc rb r", p=P, rb=4))

            qkvc = {}
            for t_i, w_d in enumerate([wq_d, wk_d, wv_d]):
                wt = []
                for i in range(8):
                    w_tile = wp.tile([P, F], bf16, tag=f"w{i}", name=f"w{i}")
                    nc.sync.dma_start(out=w_tile, in_=w_d[i * P:(i + 1) * P, :])
                    wt.append(w_tile)
                if t_i == 0:
                    for hp in range(4):
                        em_load(hp)
                cc = qkvcp.tile([P, 4, F], bf16, tag=f"c{t_i}", name=f"c{t_i}")
                qkvc[t_i] = cc
                for rb in range(4):
                    for h2 in range(2):
                        ps = ppool.tile([P, 512], f32, tag="pj", name="pj")
                        for kc in range(8):
                            nc.tensor.matmul(
                                ps, xt[:, kc, rb, :],
                                wt[kc][:, 512 * h2:512 * (h2 + 1)],
                                start=(kc == 0), stop=False)
                        nc.tensor.matmul(
                            ps, ones_all[0:1, :],
                            bias_sb[0:1, 1024 * t_i + 512 * h2:1024 * t_i + 512 * h2 + 512],
                            start=False, stop=True)
                        nc.vector.tensor_copy(cc[:, rb, 512 * h2:512 * (h2 + 1)], ps)

            # V shuffle through DRAM into [k'-chunk partitions, d] layout (+ones)
            vs = dramp.tile([RW, F], bf16, tag="vs", name="vs")
            nc.sync.dma_start(out=vs.rearrange("(rb p) c -> p rb c", p=P),
                              in_=qkvc[2])
            for hp in range(HG):
                e = hp % 2
                nc.vector.memset(V2[hp], 0.0)
                dcol = 64 * e
                ones_col = 64 * (1 - e)
                src = vs[64 * hp:64 * hp + 64, :].rearrange(
                    "(j r) (cb d) -> (r cb) j d", j=8, cb=16)
                nc.sync.dma_start(out=V2[hp][:, :, dcol:dcol + 64], in_=src)
                nc.vector.memset(V2[hp][:, :, ones_col:ones_col + 1], 1.0)

            # round 0 softmax-1: sums right after proj (PE stays warm),
            # recips/broadcasts flow on ACT/DVE/GPSIMD under the transposes
            with tc.tile_pool(name="r1ps0", bufs=2, space="PSUM") as r1psp0:
                softmax1_sums([0, 1, 2, 3], r1psp0)
            softmax1_finish([0, 1, 2, 3])

            # Q^T / K^T via 64x64 PE transposes
            for t_i, dstmap in ((0, QT), (1, KT)):
                for hp in range(HG):
                    pr, e = hp // 2, hp % 2
                    base = 64 * e
                    src_rows = qkvc[t_i][base:base + 64, pr, :]
                    for half in range(2):
                        pst = tpool.tile([64, 512], bf16, tag="tp", name="tp")
                        for cb8 in range(8):
                            cb = 8 * half + cb8
                            nc.tensor.transpose(
                                pst[0:64, 64 * cb8:64 * cb8 + 64],
                                src_rows.rearrange("p (cb d) -> p cb d", cb=16)[:, cb, :],
                                ident[base:base + 64, base:base + 64])
                        dst = dstmap[pr][base:base + 64, :].rearrange(
                            "p (r cb) -> p r cb", cb=16)[:, :, 8 * half:8 * half + 8]
                        nc.vector.tensor_copy(
                            dst, pst[0:64, :].rearrange("p (cb8 r) -> p r cb8", cb8=8))

        # ---------- phase 2: attention, two rounds of 4 heads ----------
        for rnd in range(2):
            hps = list(range(4 * rnd, 4 * rnd + 4))
            if rnd == 1:
                with tc.tile_pool(name="r1ps1", bufs=2, space="PSUM") as r1psp:
                    softmax1_sums(hps, r1psp)
                softmax1_finish(hps)
                # round-0 half of the output projection: fills the PE while
                # the round-1 reciprocal chain runs on ACT/DVE/GPSIMD, and
                # halves the serial outproj tail after round 1
                with tc.tile_pool(name="opsA", bufs=2, space="PSUM") as opsA:
                    for fo in range(8):
                        oa = oap.tile([P, S], bf16, tag=f"oa{fo}", name=f"oa{fo}")
                        oA[fo] = oa
                        for h2 in range(2):
                            ps = opsA.tile([P, 512], f32, tag="opA", name="opA")
                            for pr in range(2):
                                nc.tensor.matmul(
                                    ps, wo_sb[:, pr, 128 * fo:128 * (fo + 1)],
                                    OT[pr][:, 512 * h2:512 * (h2 + 1)],
                                    start=(pr == 0), stop=(pr == 1))
                            nc.vector.tensor_copy(
                                oa[:, 512 * h2:512 * (h2 + 1)], ps)
            # 2d/2e: attention + PV + softmax-2 normalize
            with tc.tile_pool(name=f"ep{rnd}", bufs=4) as epool, \
                    tc.tile_pool(name=f"qk{rnd}", bufs=2, space="PSUM") as qkps, \
                    tc.tile_pool(name=f"pv{rnd}", bufs=2, space="PSUM") as pvps:
                def attention_pair(hpA, pvA, hpB, pvB):
                    # two heads' j-loops interleaved, with each head's PV
                    # pipelined one j behind its QK: the PE always has the
                    # sibling head's matmuls in its queue while one head's
                    # E-path (DVE mul/add + ACT exp) completes, keeping PE
                    # duty high enough to hold the HAM clock at 2.4 GHz
                    pair = ((hpA, pvA), (hpB, pvB))
                    lastE = {}

                    def emit_qk(hp, j):
                        pr, e = hp // 2, hp % 2
                        base = 64 * e
                        qk = qkps.tile([P, S], f32, tag="qk", name="qk")
                        for h2 in range(2):
                            nc.tensor.matmul(
                                qk[:, 512 * h2:512 * (h2 + 1)],
                                KT[pr][base:base + 64, 128 * j:128 * (j + 1)],
                                QT[pr][base:base + 64, 512 * h2:512 * (h2 + 1)],
                                start=True, stop=True)
                        return qk

                    def emit_epath(hp, j, qk):
                        jl = 128 * j if causal else 0
                        Ej = epool.tile([P, S], bf16, tag="E", name="E")
                        if jl < S:
                            nc.vector.tensor_mul(
                                Ej[:, jl:], eM[hp][:, woff[j]:woff[j] + S - jl],
                                r1bc[hp][:, jl:])
                            nc.vector.tensor_add(qk[:, jl:], qk[:, jl:], Ej[:, jl:])
                        nc.scalar.activation(Ej, qk, Exp)
                        lastE[hp] = Ej

                    def emit_pv(hp, pv, j):
                        for h2 in range(2):
                            nc.tensor.matmul(
                                pv[:, 512 * h2:512 * (h2 + 1)],
                                V2[hp][:, j, :],
                                lastE[hp][:, 512 * h2:512 * (h2 + 1)],
                                start=(j == 0), stop=(j == 7))

                    for j in range(8):
                        qks = {hp: emit_qk(hp, j) for hp, _ in pair}
                        if j > 0:
                            for hp, pv in pair:
                                emit_pv(hp, pv, j - 1)
                        for hp, _ in pair:
                            emit_epath(hp, j, qks[hp])
                    for hp, pv in pair:
                        emit_pv(hp, pv, 7)
                # normalize rows of PV by 1/rowsum2 (ones column) in head
                # PAIRS: Ln,Ln then Exp,Exp keeps the ACT table reloads to 2
                # per pair and leaves the hp -> hp+1 boundary load-free, so
                # the next head's E-exps (and the PE behind them) never stall
                # on a table switch. DVE stages the sum row to partition 0,
                # GPSIMD broadcasts the reciprocal.
                def normalize_pair(hpA, pvA, hpB, pvB):
                    tiles = {}
                    for hp, pv in ((hpA, pvA), (hpB, pvB)):
                        e = hp % 2
                        sum_row = 64 * (1 - e)
                        r2f = miscp.tile([1, S], f32, tag=f"r2f{e}", name=f"r2f{e}")
                        r2s = miscp.tile([1, S], bf16, tag=f"r2s{e}", name=f"r2s{e}")
                        r2b = miscp.tile([P, S], bf16, tag=f"r2b{e}", name=f"r2b{e}")
                        tiles[hp] = (pv, r2f, r2s, r2b)
                        nc.vector.tensor_copy(r2f, pv[sum_row:sum_row + 1, :])
                    for hp in (hpA, hpB):
                        pv, r2f, r2s, r2b = tiles[hp]
                        nc.scalar.activation(r2f, r2f, Ln)
                    for hp in (hpA, hpB):
                        pv, r2f, r2s, r2b = tiles[hp]
                        nc.scalar.activation(r2s, r2f, Exp, scale=-1.0)
                    for hp in (hpA, hpB):
                        pv, r2f, r2s, r2b = tiles[hp]
                        pr, e = hp // 2, hp % 2
                        dlo = 64 * e
                        nc.gpsimd.partition_broadcast(r2b, r2s)
                        nc.vector.tensor_mul(OT[pr][dlo:dlo + 64, :],
                                             pv[dlo:dlo + 64, :],
                                             r2b[dlo:dlo + 64, :])

                for pi in range(2):
                    hpA, hpB = hps[2 * pi], hps[2 * pi + 1]
                    pvA = pvps.tile([P, S], f32, tag="pv", name="pv")
                    pvB = pvps.tile([P, S], f32, tag="pv", name="pv")
                    attention_pair(hpA, pvA, hpB, pvB)
                    normalize_pair(hpA, pvA, hpB, pvB)
                    if rnd == 0:
                        em_load(hpA + 4)
                        em_load(hpB + 4)

        # ---------- phase 3: partial output projection ----------
        with tc.tile_pool(name="os", bufs=3) as osp, \
                tc.tile_pool(name="ops", bufs=2, space="PSUM") as opsum:
            for fo in range(8):
                ot = osp.tile([P, S], bf16, tag="os", name="os")
                for h2 in range(2):
                    ps = opsum.tile([P, 512], f32, tag="op", name="op")
                    for pr in (2, 3):
                        nc.tensor.matmul(ps, wo_sb[:, pr, 128 * fo:128 * (fo + 1)],
                                         OT[pr][:, 512 * h2:512 * (h2 + 1)],
                                         start=(pr == 2), stop=(pr == 3))
                    nc.vector.tensor_add(ot[:, 512 * h2:512 * (h2 + 1)], ps,
                                         oA[fo][:, 512 * h2:512 * (h2 + 1)])
                nc.sync.dma_start(out=out_d[128 * fo:128 * (fo + 1), :], in_=ot)

    nc.compile()
    return nc


def _prep_host(x, str_mat, attn_mask, Wq, bq, Wk, bk, Wv, bv, Wo, bo):
    bf = _np_bf16()
    x = np.asarray(x, np.float32)
    str_mat = np.asarray(str_mat, np.float32)
    attn_mask = np.asarray(attn_mask, np.float32)
    mask = attn_mask[:, 0]  # [b, s, s]
    causal = bool((mask == np.tril(np.ones((S, S), np.float32))[None]).all())
    strT = np.where(mask[:, None] == 0.0, NEG_FILL, str_mat).transpose(0, 1, 3, 2)
    strT = strT.astype(bf)
    xT = x.transpose(0, 2, 1).astype(bf)  # [b, f, s]
    Wq_s = (np.asarray(Wq, np.float32) / D).astype(bf)
    bq_s = (np.asarray(bq, np.float32) / D)
    bias = np.stack([bq_s, np.asarray(bk, np.float32),
                     np.asarray(bv, np.float32)]).astype(bf)
    Wk_c = np.asarray(Wk, np.float32).astype(bf)
    Wv_c = np.asarray(Wv, np.float32).astype(bf)
    Wo_c = np.asarray(Wo, np.float32).astype(bf)
    ident = np.eye(P, dtype=np.float32).astype(bf)
    in_maps = []
    for c in range(NCORES):
        bc, g = c // 2, c % 2
        in_maps.append({
            "xT": np.ascontiguousarray(xT[bc, :, RW * g:RW * (g + 1)]),
            "strT": np.ascontiguousarray(strT[bc, HG * g:HG * (g + 1)]),
            "wq": Wq_s, "wk": Wk_c, "wv": Wv_c,
            "wo": np.ascontiguousarray(Wo_c[RW * g:RW * (g + 1)]),
            "bqkv": bias, "ident": ident,
        })
    return in_maps, causal


def kernel(**inputs):
    from concourse.bass_utils import run_bass_kernel_spmd

    in_maps, causal = _prep_host(**inputs)
    key = ("v2", causal)
    if key not in _CACHE:
        _CACHE[key] = _build_nc(causal=causal)
    nc = _CACHE[key]
    res = run_bass_kernel_spmd(nc, in_maps, core_ids=list(range(NCORES)))
    partials = [np.asarray(r["outT"], np.float32) for r in res.results]
    out = np.stack([partials[2 * bc] + partials[2 * bc + 1] for bc in range(B)])
    out = out.transpose(0, 2, 1) + np.asarray(bo_arr(inputs), np.float32)
    return np.ascontiguousarray(out.astype(np.float32))


def bo_arr(inputs):
    return np.asarray(inputs["bo"], np.float32)


# revision 29
# speedup vs baseline: 1.0417x; 1.0174x over previous
"""Trainium2 Bass kernel for nn_MHInrAttn (sparse_attention, b=4 s=1024 f=1024 h=16).

v2 sharding (8 NeuronCores): core c -> (batch c//2, head-group c%2).
Each core: 8 heads of one batch. The reference uses a raw .reshape with NO
transpose, so head h's Q/K/V come from rows [64h, 64h+64) of the projected
[s, f] matrix -> a head-group needs only x rows [512g, 512g+512).

Per core: project Q/K/V for 512 rows, attention for 8 heads in scores^T
[k, q] orientation, partial output projection through its 512 Wo rows.
Host: sum partial pairs per batch, transpose, add bo.

Device details:
  - all matmul operands bf16 (1 cyc/row on PE), fp32 PSUM accumulation
  - str^T streams causally trimmed into a PACKED eM tile (one wide exp/head)
  - softmax-1 k-sums via ones-column PE matmul; 1/x via Ln->Exp on ACT with
    Ln/Exp batched per phase (they live in different ACT table sets)
  - qk pair tile [128, 1024] spans 2 PSUM banks -> wide ACT/DVE ops
  - PV carries a ones column in V for softmax-2 row sums for free
"""

import numpy as np

B, S, F, H, D = 4, 1024, 1024, 16, 64
NCORES = 8
HG = 8           # heads per core (head-group)
RW = 512         # proj rows per core
P = 128
NEG_FILL = -40.0

_CACHE = {}

_WOFF = [0, 1024, 1920, 2688, 3328, 3840, 4224, 4480]  # packed eM offsets
_WTOT = 4608


def _np_bf16():
    import ml_dtypes

    return ml_dtypes.bfloat16


def _build_nc(causal=True):
    from contextlib import ExitStack

    import concourse.bacc as bacc
    import concourse.tile as tile
    from concourse import mybir

    dt = mybir.dt
    f32 = dt.float32
    bf16 = dt.bfloat16
    Exp = mybir.ActivationFunctionType.Exp
    Ln = mybir.ActivationFunctionType.Ln

    nc = bacc.Bacc("TRN2", target_bir_lowering=False, debug=False)

    xT_d = nc.dram_tensor("xT", [F, RW], bf16, kind="ExternalInput").ap()
    str_d = nc.dram_tensor("strT", [HG, S, S], bf16, kind="ExternalInput").ap()
    wq_d = nc.dram_tensor("wq", [F, F], bf16, kind="ExternalInput").ap()
    wk_d = nc.dram_tensor("wk", [F, F], bf16, kind="ExternalInput").ap()
    wv_d = nc.dram_tensor("wv", [F, F], bf16, kind="ExternalInput").ap()
    wo_d = nc.dram_tensor("wo", [RW, F], bf16, kind="ExternalInput").ap()
    bias_d = nc.dram_tensor("bqkv", [3, F], bf16, kind="ExternalInput").ap()
    ident_d = nc.dram_tensor("ident", [P, P], bf16, kind="ExternalInput").ap()
    out_d = nc.dram_tensor("outT", [F, S], bf16, kind="ExternalOutput").ap()

    woff = _WOFF if causal else [1024 * j for j in range(8)]
    wtot = _WTOT if causal else 8 * 1024

    with ExitStack() as ctx:
        tc = ctx.enter_context(tile.TileContext(nc))
        consts = ctx.enter_context(tc.tile_pool(name="consts", bufs=1))
        qtkt = ctx.enter_context(tc.tile_pool(name="qtkt", bufs=1))
        v2p = ctx.enter_context(tc.tile_pool(name="v2", bufs=1))
        outp = ctx.enter_context(tc.tile_pool(name="outp", bufs=1))
        wop = ctx.enter_context(tc.tile_pool(name="wop", bufs=1))
        dramp = ctx.enter_context(tc.tile_pool(name="dram", bufs=1, space="DRAM"))

        ident = consts.tile([P, P], bf16, tag="ident", name="ident")
        nc.sync.dma_start(out=ident, in_=ident_d)
        ones_all = consts.tile([P, P], bf16, tag="ones", name="ones")
        nc.vector.memset(ones_all, 1.0)
        bias_sb = consts.tile([1, 3 * F], bf16, tag="bias", name="bias")
        nc.sync.dma_start(out=bias_sb, in_=bias_d.rearrange("a b -> (a b)").unsqueeze(0))
        wo_sb = wop.tile([P, 4, F], bf16, tag="wo", name="wo")
        nc.sync.dma_start(out=wo_sb, in_=wo_d.rearrange("(c p) f -> p c f", p=P))

        QT, KT, OT, V2 = {}, {}, {}, {}
        for pr in range(4):  # head pairs
            QT[pr] = qtkt.tile([P, S], bf16, tag=f"qt{pr}", name=f"qt{pr}")
            KT[pr] = qtkt.tile([P, S], bf16, tag=f"kt{pr}", name=f"kt{pr}")
            OT[pr] = outp.tile([P, S], bf16, tag=f"ot{pr}", name=f"ot{pr}")
        for hp in range(HG):
            V2[hp] = v2p.tile([P, 8, P], bf16, tag=f"v{hp}", name=f"v{hp}")

        # ---------- phase 2 setup: str loads + exps for round 0 ----------
        # emitted before phase 1 in the ACT/DMA streams: the scalar engine is
        # otherwise idle during the projections, so round 0's eM exps are done
        # by the time the PE finishes the transposes
        emp = ctx.enter_context(tc.tile_pool(name="em", bufs=1))
        oap = ctx.enter_context(tc.tile_pool(name="oA", bufs=1))
        oA = {}
        rbcp = ctx.enter_context(tc.tile_pool(name="rbc", bufs=1))
        miscp = ctx.enter_context(tc.tile_pool(name="m", bufs=1))
        eM, r1bc, r1raw, r1sb = {}, {}, {}, {}

        def em_load(hp):
            t = emp.tile([P, wtot], bf16, tag=f"e{hp % 4}", name=f"e{hp % 4}")
            eM[hp] = t
            for j in range(8):
                jl = 128 * j if causal else 0
                w = S - jl
                nc.sync.dma_start(out=t[:, woff[j]:woff[j] + w],
                                  in_=str_d[hp, 128 * j:128 * (j + 1), jl:])
            nc.scalar.activation(t, t, Exp)

        # round 0's eM loads are emitted just after the xt/wq DMAs below (str
        # must not queue ahead of the DMAs the projections wait on); round 1's
        # are prefetched inside round 0's attention loop.

        def softmax1_sums(hps, r1psp):
            # softmax-1 denominators via ones-column matmuls
            for hp in hps:
                ps_r1 = r1psp.tile([1, S], f32, tag="r1", name="r1")
                for j in range(8):
                    jl = 128 * j if causal else 0
                    for h2 in range(2):
                        lo = max(512 * h2, jl)
                        hi = 512 * (h2 + 1)
                        if lo < hi:
                            last_j = (3 if h2 == 0 else 7) if causal else 7
                            nc.tensor.matmul(
                                ps_r1[0:1, lo:hi],
                                ones_all[:, 0:1],
                                eM[hp][:, woff[j] + lo - jl:woff[j] + hi - jl],
                                start=(j == 0), stop=(j == last_j))
                rr = miscp.tile([1, S], f32, tag=f"rr{hp % 4}", name=f"rr{hp % 4}")
                r1raw[hp] = rr
                nc.vector.tensor_copy(rr, ps_r1)

        def softmax1_finish(hps):
            # reciprocals: the round's first head goes solo (its 1/R1 gates
            # the first attention head's E-path), the rest batch their
            # Lns/Exps to limit ACT table reloads; the idle GPSIMD engine
            # broadcasts (r1sb is a partition-0 [1, S] tile)
            def recip_block(block):
                for hp in block:
                    nc.scalar.activation(r1raw[hp], r1raw[hp], Ln)
                for hp in block:
                    rs = miscp.tile([1, S], bf16, tag=f"rs{hp % 4}", name=f"rs{hp % 4}")
                    r1sb[hp] = rs
                    nc.scalar.activation(rs, r1raw[hp], Exp, scale=-1.0)
            recip_block(hps[:1])
            recip_block(hps[1:])
            for hp in hps:
                rbc = rbcp.tile([P, S], bf16, tag=f"rb{hp % 4}", name=f"rb{hp % 4}")
                r1bc[hp] = rbc
                nc.gpsimd.partition_broadcast(rbc, r1sb[hp])


        # ---------- phase 1: projections + layout shuffles ----------
        with tc.tile_pool(name="xt", bufs=1) as xtp, \
                tc.tile_pool(name="wpool", bufs=1) as wp, \
                tc.tile_pool(name="qkvc", bufs=1) as qkvcp, \
                tc.tile_pool(name="pj", bufs=2, space="PSUM") as ppool, \
                tc.tile_pool(name="tp", bufs=2, space="PSUM") as tpool:
            xt = xtp.tile([P, 8, 4, P], bf16, tag="xt", name="xt")
            nc.sync.dma_start(
                out=xt, in_=xT_d.rearrange("(kc p) (rb r) -> p kc rb r", p=P, rb=4))

            qkvc = {}
            for t_i, w_d in enumerate([wq_d, wk_d, wv_d]):
                wt = []
                for i in range(8):
                    w_tile = wp.tile([P, F], bf16, tag=f"w{i}", name=f"w{i}")
                    nc.sync.dma_start(out=w_tile, in_=w_d[i * P:(i + 1) * P, :])
                    wt.append(w_tile)
                if t_i == 0:
                    for hp in range(4):
                        em_load(hp)
                cc = qkvcp.tile([P, 4, F], bf16, tag=f"c{t_i}", name=f"c{t_i}")
                qkvc[t_i] = cc
                for rb in range(4):
                    for h2 in range(2):
                        ps = ppool.tile([P, 512], f32, tag="pj", name="pj")
                        for kc in range(8):
                            nc.tensor.matmul(
                                ps, xt[:, kc, rb, :],
                                wt[kc][:, 512 * h2:512 * (h2 + 1)],
                                start=(kc == 0), stop=False)
                        nc.tensor.matmul(
                            ps, ones_all[0:1, :],
                            bias_sb[0:1, 1024 * t_i + 512 * h2:1024 * t_i + 512 * h2 + 512],
                            start=False, stop=True)
                        nc.vector.tensor_copy(cc[:, rb, 512 * h2:512 * (h2 + 1)], ps)

            # V shuffle through DRAM into [k'-chunk partitions, d] layout (+ones)
            vs = dramp.tile([RW, F], bf16, tag="vs", name="vs")
            nc.sync.dma_start(out=vs.rearrange("(rb p) c -> p rb c", p=P),
                              in_=qkvc[2])
            for hp in range(HG):
                e = hp % 2
                nc.vector.memset(V2[hp], 0.0)
                dcol = 64 * e
                ones_col = 64 * (1 - e)
                src = vs[64 * hp:64 * hp + 64, :].rearrange(
                    "(j r) (cb d) -> (r cb) j d", j=8, cb=16)
                nc.sync.dma_start(out=V2[hp][:, :, dcol:dcol + 64], in_=src)
                nc.vector.memset(V2[hp][:, :, ones_col:ones_col + 1], 1.0)

            # round 0 softmax-1: sums right after proj (PE stays warm),
            # recips/broadcasts flow on ACT/DVE/GPSIMD under the transposes
            with tc.tile_pool(name="r1ps0", bufs=2, space="PSUM") as r1psp0:
                softmax1_sums([0, 1, 2, 3], r1psp0)
            softmax1_finish([0, 1, 2, 3])

            # Q^T / K^T via 64x64 PE transposes
            for t_i, dstmap in ((0, QT), (1, KT)):
                for hp in range(HG):
                    pr, e = hp // 2, hp % 2
                    base = 64 * e
                    src_rows = qkvc[t_i][base:base + 64, pr, :]
                    for half in range(2):
                        pst = tpool.tile([64, 512], bf16, tag="tp", name="tp")
                        for cb8 in range(8):
                            cb = 8 * half + cb8
                            nc.tensor.transpose(
                                pst[0:64, 64 * cb8:64 * cb8 + 64],
                                src_rows.rearrange("p (cb d) -> p cb d", cb=16)[:, cb, :],
                                ident[base:base + 64, base:base + 64])
                        dst = dstmap[pr][base:base + 64, :].rearrange(
                            "p (r cb) -> p r cb", cb=16)[:, :, 8 * half:8 * half + 8]
                        nc.vector.tensor_copy(
                            dst, pst[0:64, :].rearrange("p (cb8 r) -> p r cb8", cb8=8))

        # ---------- phase 2: attention, two rounds of 4 heads ----------
        for rnd in range(2):
            hps = list(range(4 * rnd, 4 * rnd + 4))
            if rnd == 1:
                with tc.tile_pool(name="r1ps1", bufs=2, space="PSUM") as r1psp:
                    softmax1_sums(hps, r1psp)
                softmax1_finish(hps)
                # round-0 half of the output projection: fills the PE while
                # the round-1 reciprocal chain runs on ACT/DVE/GPSIMD, and
                # halves the serial outproj tail after round 1
                with tc.tile_pool(name="opsA", bufs=2, space="PSUM") as opsA:
                    for fo in range(8):
                        oa = oap.tile([P, S], bf16, tag=f"oa{fo}", name=f"oa{fo}")
                        oA[fo] = oa
                        for h2 in range(2):
                            ps = opsA.tile([P, 512], f32, tag="opA", name="opA")
                            for pr in range(2):
                                nc.tensor.matmul(
                                    ps, wo_sb[:, pr, 128 * fo:128 * (fo + 1)],
                                    OT[pr][:, 512 * h2:512 * (h2 + 1)],
                                    start=(pr == 0), stop=(pr == 1))
                            nc.vector.tensor_copy(
                                oa[:, 512 * h2:512 * (h2 + 1)], ps)
            # 2d/2e: attention + PV + softmax-2 normalize
            with tc.tile_pool(name=f"ep{rnd}", bufs=4) as epool, \
                    tc.tile_pool(name=f"qk{rnd}", bufs=2, space="PSUM") as qkps, \
                    tc.tile_pool(name=f"pv{rnd}", bufs=2, space="PSUM") as pvps:
                def attention_pair(hpA, pvA, hpB, pvB, fillers=None):
                    # two heads' j-loops interleaved, with each head's PV
                    # pipelined one j behind its QK: the PE always has the
                    # sibling head's matmuls in its queue while one head's
                    # E-path (DVE mul/add + ACT exp) completes, keeping PE
                    # duty high enough to hold the HAM clock at 2.4 GHz
                    pair = ((hpA, pvA), (hpB, pvB))
                    lastE = {}

                    def emit_qk(hp, j):
                        pr, e = hp // 2, hp % 2
                        base = 64 * e
                        qk = qkps.tile([P, S], f32, tag="qk", name="qk")
                        for h2 in range(2):
                            nc.tensor.matmul(
                                qk[:, 512 * h2:512 * (h2 + 1)],
                                KT[pr][base:base + 64, 128 * j:128 * (j + 1)],
                                QT[pr][base:base + 64, 512 * h2:512 * (h2 + 1)],
                                start=True, stop=True)
                        return qk

                    def emit_epath(hp, j, qk):
                        jl = 128 * j if causal else 0
                        Ej = epool.tile([P, S], bf16, tag="E", name="E")
                        if jl < S:
                            nc.vector.tensor_mul(
                                Ej[:, jl:], eM[hp][:, woff[j]:woff[j] + S - jl],
                                r1bc[hp][:, jl:])
                            nc.vector.tensor_add(qk[:, jl:], qk[:, jl:], Ej[:, jl:])
                        nc.scalar.activation(Ej, qk, Exp)
                        lastE[hp] = Ej

                    def emit_pv(hp, pv, j):
                        for h2 in range(2):
                            nc.tensor.matmul(
                                pv[:, 512 * h2:512 * (h2 + 1)],
                                V2[hp][:, j, :],
                                lastE[hp][:, 512 * h2:512 * (h2 + 1)],
                                start=(j == 0), stop=(j == 7))

                    for j in range(8):
                        qks = {hp: emit_qk(hp, j) for hp, _ in pair}
                        if j > 0:
                            for hp, pv in pair:
                                emit_pv(hp, pv, j - 1)
                        for hp, _ in pair:
                            emit_epath(hp, j, qks[hp])
                        if fillers and j in fillers:
                            fillers[j]()
                    for hp, pv in pair:
                        emit_pv(hp, pv, 7)
                # normalize rows of PV by 1/rowsum2 (ones column) in head
                # PAIRS: Ln,Ln then Exp,Exp keeps the ACT table reloads to 2
                # per pair and leaves the hp -> hp+1 boundary load-free, so
                # the next head's E-exps (and the PE behind them) never stall
                # on a table switch. DVE stages the sum row to partition 0,
                # GPSIMD broadcasts the reciprocal.
                def normalize_pair(hpA, pvA, hpB, pvB):
                    tiles = {}
                    for hp, pv in ((hpA, pvA), (hpB, pvB)):
                        e = hp % 2
                        sum_row = 64 * (1 - e)
                        r2f = miscp.tile([1, S], f32, tag=f"r2f{e}", name=f"r2f{e}")
                        r2s = miscp.tile([1, S], bf16, tag=f"r2s{e}", name=f"r2s{e}")
                        r2b = miscp.tile([P, S], bf16, tag=f"r2b{e}", name=f"r2b{e}")
                        tiles[hp] = (pv, r2f, r2s, r2b)
                        nc.vector.tensor_copy(r2f, pv[sum_row:sum_row + 1, :])
                    for hp in (hpA, hpB):
                        pv, r2f, r2s, r2b = tiles[hp]
                        nc.scalar.activation(r2f, r2f, Ln)
                    for hp in (hpA, hpB):
                        pv, r2f, r2s, r2b = tiles[hp]
                        nc.scalar.activation(r2s, r2f, Exp, scale=-1.0)
                    for hp in (hpA, hpB):
                        pv, r2f, r2s, r2b = tiles[hp]
                        pr, e = hp // 2, hp % 2
                        dlo = 64 * e
                        nc.gpsimd.partition_broadcast(r2b, r2s)
                        nc.vector.tensor_mul(OT[pr][dlo:dlo + 64, :],
                                             pv[dlo:dlo + 64, :],
                                             r2b[dlo:dlo + 64, :])

                for pi in range(2):
                    hpA, hpB = hps[2 * pi], hps[2 * pi + 1]
                    pvA = pvps.tile([P, S], f32, tag="pv", name="pv")
                    pvB = pvps.tile([P, S], f32, tag="pv", name="pv")
                    # round-1 eM prefetch: heads 4/5 thread INTO pair 1's
                    # j-loop (their DMAs are tag-gated on pair 0's last read;
                    # mid-loop ACT slack absorbs the 4us exps instead of
                    # stalling the PE at the pair boundary); heads 6/7 must
                    # wait for pair 1's own tags, so they stay at round end
                    fillers = None
                    if rnd == 0 and pi == 1:
                        fillers = {0: (lambda: em_load(4)), 4: (lambda: em_load(5))}
                    attention_pair(hpA, pvA, hpB, pvB, fillers)
                    normalize_pair(hpA, pvA, hpB, pvB)
                    if rnd == 0 and pi == 1:
                        em_load(6)
                        em_load(7)

        # ---------- phase 3: partial output projection ----------
        with tc.tile_pool(name="os", bufs=3) as osp, \
                tc.tile_pool(name="ops", bufs=2, space="PSUM") as opsum:
            for fo in range(8):
                ot = osp.tile([P, S], bf16, tag="os", name="os")
                for h2 in range(2):
                    ps = opsum.tile([P, 512], f32, tag="op", name="op")
                    for pr in (2, 3):
                        nc.tensor.matmul(ps, wo_sb[:, pr, 128 * fo:128 * (fo + 1)],
                                         OT[pr][:, 512 * h2:512 * (h2 + 1)],
                                         start=(pr == 2), stop=(pr == 3))
                    nc.vector.tensor_add(ot[:, 512 * h2:512 * (h2 + 1)], ps,
                                         oA[fo][:, 512 * h2:512 * (h2 + 1)])
                nc.sync.dma_start(out=out_d[128 * fo:128 * (fo + 1), :], in_=ot)

    nc.compile()
    return nc


def _prep_host(x, str_mat, attn_mask, Wq, bq, Wk, bk, Wv, bv, Wo, bo):
    bf = _np_bf16()
    x = np.asarray(x, np.float32)
    str_mat = np.asarray(str_mat, np.float32)
    attn_mask = np.asarray(attn_mask, np.float32)
    mask = attn_mask[:, 0]  # [b, s, s]
    causal = bool((mask == np.tril(np.ones((S, S), np.float32))[None]).all())
    strT = np.where(mask[:, None] == 0.0, NEG_FILL, str_mat).transpose(0, 1, 3, 2)
    strT = strT.astype(bf)
    xT = x.transpose(0, 2, 1).astype(bf)  # [b, f, s]
    Wq_s = (np.asarray(Wq, np.float32) / D).astype(bf)
    bq_s = (np.asarray(bq, np.float32) / D)
    bias = np.stack([bq_s, np.asarray(bk, np.float32),
                     np.asarray(bv, np.float32)]).astype(bf)
    Wk_c = np.asarray(Wk, np.float32).astype(bf)
    Wv_c = np.asarray(Wv, np.float32).astype(bf)
    Wo_c = np.asarray(Wo, np.float32).astype(bf)
    ident = np.eye(P, dtype=np.float32).astype(bf)
    in_maps = []
    for c in range(NCORES):
        bc, g = c // 2, c % 2
        in_maps.append({
            "xT": np.ascontiguousarray(xT[bc, :, RW * g:RW * (g + 1)]),
            "strT": np.ascontiguousarray(strT[bc, HG * g:HG * (g + 1)]),
            "wq": Wq_s, "wk": Wk_c, "wv": Wv_c,
            "wo": np.ascontiguousarray(Wo_c[RW * g:RW * (g + 1)]),
            "bqkv": bias, "ident": ident,
        })
    return in_maps, causal


def kernel(**inputs):
    from concourse.bass_utils import run_bass_kernel_spmd

    in_maps, causal = _prep_host(**inputs)
    key = ("v2", causal)
    if key not in _CACHE:
        _CACHE[key] = _build_nc(causal=causal)
    nc = _CACHE[key]
    res = run_bass_kernel_spmd(nc, in_maps, core_ids=list(range(NCORES)))
    partials = [np.asarray(r["outT"], np.float32) for r in res.results]
    out = np.stack([partials[2 * bc] + partials[2 * bc + 1] for bc in range(B)])
    out = out.transpose(0, 2, 1) + np.asarray(bo_arr(inputs), np.float32)
    return np.ascontiguousarray(out.astype(np.float32))


def bo_arr(inputs):
    return np.asarray(inputs["bo"], np.float32)


# revision 30
# speedup vs baseline: 1.0545x; 1.0122x over previous
"""Trainium2 Bass kernel for nn_MHInrAttn (sparse_attention, b=4 s=1024 f=1024 h=16).

v2 sharding (8 NeuronCores): core c -> (batch c//2, head-group c%2).
Each core: 8 heads of one batch. The reference uses a raw .reshape with NO
transpose, so head h's Q/K/V come from rows [64h, 64h+64) of the projected
[s, f] matrix -> a head-group needs only x rows [512g, 512g+512).

Per core: project Q/K/V for 512 rows, attention for 8 heads in scores^T
[k, q] orientation, partial output projection through its 512 Wo rows.
Host: sum partial pairs per batch, transpose, add bo.

Device details:
  - all matmul operands bf16 (1 cyc/row on PE), fp32 PSUM accumulation
  - str^T streams causally trimmed into a PACKED eM tile (one wide exp/head)
  - softmax-1 k-sums via ones-column PE matmul; 1/x via Ln->Exp on ACT with
    Ln/Exp batched per phase (they live in different ACT table sets)
  - qk pair tile [128, 1024] spans 2 PSUM banks -> wide ACT/DVE ops
  - PV carries a ones column in V for softmax-2 row sums for free
"""

import numpy as np

B, S, F, H, D = 4, 1024, 1024, 16, 64
NCORES = 8
HG = 8           # heads per core (head-group)
RW = 512         # proj rows per core
P = 128
NEG_FILL = -40.0

_CACHE = {}

_WOFF = [0, 1024, 1920, 2688, 3328, 3840, 4224, 4480]  # packed eM offsets
_WTOT = 4608


def _np_bf16():
    import ml_dtypes

    return ml_dtypes.bfloat16


def _build_nc(causal=True):
    from contextlib import ExitStack

    import concourse.bacc as bacc
    import concourse.tile as tile
    from concourse import mybir

    dt = mybir.dt
    f32 = dt.float32
    bf16 = dt.bfloat16
    Exp = mybir.ActivationFunctionType.Exp
    Ln = mybir.ActivationFunctionType.Ln

    nc = bacc.Bacc("TRN2", target_bir_lowering=False, debug=False)

    xT_d = nc.dram_tensor("xT", [F, RW], bf16, kind="ExternalInput").ap()
    str_d = nc.dram_tensor("strT", [HG, S, S], bf16, kind="ExternalInput").ap()
    wq_d = nc.dram_tensor("wq", [F, F], bf16, kind="ExternalInput").ap()
    wk_d = nc.dram_tensor("wk", [F, F], bf16, kind="ExternalInput").ap()
    wv_d = nc.dram_tensor("wv", [F, F], bf16, kind="ExternalInput").ap()
    wo_d = nc.dram_tensor("wo", [RW, F], bf16, kind="ExternalInput").ap()
    bias_d = nc.dram_tensor("bqkv", [3, F], bf16, kind="ExternalInput").ap()
    ident_d = nc.dram_tensor("ident", [P, P], bf16, kind="ExternalInput").ap()
    out_d = nc.dram_tensor("outT", [F, S], bf16, kind="ExternalOutput").ap()

    woff = _WOFF if causal else [1024 * j for j in range(8)]
    wtot = _WTOT if causal else 8 * 1024

    with ExitStack() as ctx:
        tc = ctx.enter_context(tile.TileContext(nc))
        consts = ctx.enter_context(tc.tile_pool(name="consts", bufs=1))
        qtkt = ctx.enter_context(tc.tile_pool(name="qtkt", bufs=1))
        v2p = ctx.enter_context(tc.tile_pool(name="v2", bufs=1))
        outp = ctx.enter_context(tc.tile_pool(name="outp", bufs=1))
        wop = ctx.enter_context(tc.tile_pool(name="wop", bufs=1))
        dramp = ctx.enter_context(tc.tile_pool(name="dram", bufs=1, space="DRAM"))

        ident = consts.tile([P, P], bf16, tag="ident", name="ident")
        nc.sync.dma_start(out=ident, in_=ident_d)
        ones_all = consts.tile([P, P], bf16, tag="ones", name="ones")
        nc.vector.memset(ones_all, 1.0)
        bias_sb = consts.tile([1, 3 * F], bf16, tag="bias", name="bias")
        nc.sync.dma_start(out=bias_sb, in_=bias_d.rearrange("a b -> (a b)").unsqueeze(0))
        wo_sb = wop.tile([P, 4, F], bf16, tag="wo", name="wo")
        nc.sync.dma_start(out=wo_sb, in_=wo_d.rearrange("(c p) f -> p c f", p=P))

        QT, KT, OT, V2 = {}, {}, {}, {}
        for pr in range(4):  # head pairs
            QT[pr] = qtkt.tile([P, S], bf16, tag=f"qt{pr}", name=f"qt{pr}")
            KT[pr] = qtkt.tile([P, S], bf16, tag=f"kt{pr}", name=f"kt{pr}")
            OT[pr] = outp.tile([P, S], bf16, tag=f"ot{pr}", name=f"ot{pr}")
        for hp in range(HG):
            V2[hp] = v2p.tile([P, 8, P], bf16, tag=f"v{hp}", name=f"v{hp}")

        # ---------- phase 2 setup: str loads + exps for round 0 ----------
        # emitted before phase 1 in the ACT/DMA streams: the scalar engine is
        # otherwise idle during the projections, so round 0's eM exps are done
        # by the time the PE finishes the transposes
        emp = ctx.enter_context(tc.tile_pool(name="em", bufs=1))
        oap = ctx.enter_context(tc.tile_pool(name="oA", bufs=1))
        oA = {}
        rbcp = ctx.enter_context(tc.tile_pool(name="rbc", bufs=1))
        miscp = ctx.enter_context(tc.tile_pool(name="m", bufs=1))
        eM, r1bc, r1raw, r1sb = {}, {}, {}, {}

        def em_load(hp):
            t = emp.tile([P, wtot], bf16, tag=f"e{hp % 4}", name=f"e{hp % 4}")
            eM[hp] = t
            for j in range(8):
                jl = 128 * j if causal else 0
                w = S - jl
                nc.sync.dma_start(out=t[:, woff[j]:woff[j] + w],
                                  in_=str_d[hp, 128 * j:128 * (j + 1), jl:])
            nc.scalar.activation(t, t, Exp)

        # round 0's eM loads are emitted just after the xt/wq DMAs below (str
        # must not queue ahead of the DMAs the projections wait on); round 1's
        # are prefetched inside round 0's attention loop.

        def softmax1_sums(hps, r1psp):
            # softmax-1 denominators via ones-column matmuls
            for hp in hps:
                ps_r1 = r1psp.tile([1, S], f32, tag="r1", name="r1")
                for j in range(8):
                    jl = 128 * j if causal else 0
                    for h2 in range(2):
                        lo = max(512 * h2, jl)
                        hi = 512 * (h2 + 1)
                        if lo < hi:
                            last_j = (3 if h2 == 0 else 7) if causal else 7
                            nc.tensor.matmul(
                                ps_r1[0:1, lo:hi],
                                ones_all[:, 0:1],
                                eM[hp][:, woff[j] + lo - jl:woff[j] + hi - jl],
                                start=(j == 0), stop=(j == last_j))
                rr = miscp.tile([1, S], f32, tag=f"rr{hp % 4}", name=f"rr{hp % 4}")
                r1raw[hp] = rr
                nc.vector.tensor_copy(rr, ps_r1)

        def softmax1_finish(hps):
            # reciprocals: the round's first head goes solo (its 1/R1 gates
            # the first attention head's E-path), the rest batch their
            # Lns/Exps to limit ACT table reloads; the idle GPSIMD engine
            # broadcasts (r1sb is a partition-0 [1, S] tile)
            def recip_block(block):
                for hp in block:
                    nc.scalar.activation(r1raw[hp], r1raw[hp], Ln)
                for hp in block:
                    rs = miscp.tile([1, S], bf16, tag=f"rs{hp % 4}", name=f"rs{hp % 4}")
                    r1sb[hp] = rs
                    nc.scalar.activation(rs, r1raw[hp], Exp, scale=-1.0)
            recip_block(hps[:1])
            recip_block(hps[1:])
            for hp in hps:
                rbc = rbcp.tile([P, S], bf16, tag=f"rb{hp % 4}", name=f"rb{hp % 4}")
                r1bc[hp] = rbc
                nc.gpsimd.partition_broadcast(rbc, r1sb[hp])


        # ---------- phase 1: projections + layout shuffles ----------
        with tc.tile_pool(name="xt", bufs=1) as xtp, \
                tc.tile_pool(name="wpool", bufs=1) as wp, \
                tc.tile_pool(name="qkvc", bufs=1) as qkvcp, \
                tc.tile_pool(name="pj", bufs=2, space="PSUM") as ppool, \
                tc.tile_pool(name="tp", bufs=2, space="PSUM") as tpool:
            xt = xtp.tile([P, 8, 4, P], bf16, tag="xt", name="xt")
            nc.sync.dma_start(
                out=xt, in_=xT_d.rearrange("(kc p) (rb r) -> p kc rb r", p=P, rb=4))

            qkvc = {}
            for t_i, w_d in enumerate([wq_d, wk_d, wv_d]):
                wt = []
                for i in range(8):
                    w_tile = wp.tile([P, F], bf16, tag=f"w{i}", name=f"w{i}")
                    nc.sync.dma_start(out=w_tile, in_=w_d[i * P:(i + 1) * P, :])
                    wt.append(w_tile)
                if t_i == 0:
                    for hp in range(4):
                        em_load(hp)
                cc = qkvcp.tile([P, 4, F], bf16, tag=f"c{t_i}", name=f"c{t_i}")
                qkvc[t_i] = cc
                for rb in range(4):
                    for h2 in range(2):
                        ps = ppool.tile([P, 512], f32, tag="pj", name="pj")
                        for kc in range(8):
                            nc.tensor.matmul(
                                ps, xt[:, kc, rb, :],
                                wt[kc][:, 512 * h2:512 * (h2 + 1)],
                                start=(kc == 0), stop=False)
                        nc.tensor.matmul(
                            ps, ones_all[0:1, :],
                            bias_sb[0:1, 1024 * t_i + 512 * h2:1024 * t_i + 512 * h2 + 512],
                            start=False, stop=True)
                        nc.vector.tensor_copy(cc[:, rb, 512 * h2:512 * (h2 + 1)], ps)

            # V shuffle through DRAM into [k'-chunk partitions, d] layout (+ones)
            vs = dramp.tile([RW, F], bf16, tag="vs", name="vs")
            nc.sync.dma_start(out=vs.rearrange("(rb p) c -> p rb c", p=P),
                              in_=qkvc[2])
            for hp in range(HG):
                e = hp % 2
                nc.vector.memset(V2[hp], 0.0)
                dcol = 64 * e
                ones_col = 64 * (1 - e)
                src = vs[64 * hp:64 * hp + 64, :].rearrange(
                    "(j r) (cb d) -> (r cb) j d", j=8, cb=16)
                nc.sync.dma_start(out=V2[hp][:, :, dcol:dcol + 64], in_=src)
                nc.vector.memset(V2[hp][:, :, ones_col:ones_col + 1], 1.0)

            # round 0 softmax-1: sums right after proj (PE stays warm),
            # recips/broadcasts flow on ACT/DVE/GPSIMD under the transposes
            with tc.tile_pool(name="r1ps0", bufs=2, space="PSUM") as r1psp0:
                softmax1_sums([0, 1, 2, 3], r1psp0)
            softmax1_finish([0, 1, 2, 3])

            # Q^T / K^T via 64x64 PE transposes
            for t_i, dstmap in ((0, QT), (1, KT)):
                for hp in range(HG):
                    pr, e = hp // 2, hp % 2
                    base = 64 * e
                    src_rows = qkvc[t_i][base:base + 64, pr, :]
                    for half in range(2):
                        pst = tpool.tile([64, 512], bf16, tag="tp", name="tp")
                        for cb8 in range(8):
                            cb = 8 * half + cb8
                            nc.tensor.transpose(
                                pst[0:64, 64 * cb8:64 * cb8 + 64],
                                src_rows.rearrange("p (cb d) -> p cb d", cb=16)[:, cb, :],
                                ident[base:base + 64, base:base + 64])
                        dst = dstmap[pr][base:base + 64, :].rearrange(
                            "p (r cb) -> p r cb", cb=16)[:, :, 8 * half:8 * half + 8]
                        nc.vector.tensor_copy(
                            dst, pst[0:64, :].rearrange("p (cb8 r) -> p r cb8", cb8=8))

        # ---------- phase 2: attention, two rounds of 4 heads ----------
        for rnd in range(2):
            hps = list(range(4 * rnd, 4 * rnd + 4))
            if rnd == 1:
                with tc.tile_pool(name="r1ps1", bufs=2, space="PSUM") as r1psp:
                    # heads 4/5 first (their eM is prefetched and ready); the
                    # pr0/1 outproj half then fills the PE while heads 6/7's
                    # eM exps complete, so their sums don't stall mid-stream
                    softmax1_sums([4, 5], r1psp)
                    softmax1_finish([4, 5])
                    with tc.tile_pool(name="opsA", bufs=2, space="PSUM") as opsA:
                        for fo in range(8):
                            oa = oap.tile([P, S], bf16, tag=f"oa{fo}", name=f"oa{fo}")
                            oA[fo] = oa
                            for h2 in range(2):
                                ps = opsA.tile([P, 512], f32, tag="opA", name="opA")
                                for pr in range(2):
                                    nc.tensor.matmul(
                                        ps, wo_sb[:, pr, 128 * fo:128 * (fo + 1)],
                                        OT[pr][:, 512 * h2:512 * (h2 + 1)],
                                        start=(pr == 0), stop=(pr == 1))
                                nc.vector.tensor_copy(
                                    oa[:, 512 * h2:512 * (h2 + 1)], ps)
                    softmax1_sums([6, 7], r1psp)
                    softmax1_finish([6, 7])
            # 2d/2e: attention + PV + softmax-2 normalize
            with tc.tile_pool(name=f"ep{rnd}", bufs=4) as epool, \
                    tc.tile_pool(name=f"qk{rnd}", bufs=2, space="PSUM") as qkps, \
                    tc.tile_pool(name=f"pv{rnd}", bufs=2, space="PSUM") as pvps:
                def attention_pair(hpA, pvA, hpB, pvB, fillers=None):
                    # two heads' j-loops interleaved, with each head's PV
                    # pipelined one j behind its QK: the PE always has the
                    # sibling head's matmuls in its queue while one head's
                    # E-path (DVE mul/add + ACT exp) completes, keeping PE
                    # duty high enough to hold the HAM clock at 2.4 GHz
                    pair = ((hpA, pvA), (hpB, pvB))
                    lastE = {}

                    def emit_qk(hp, j):
                        pr, e = hp // 2, hp % 2
                        base = 64 * e
                        qk = qkps.tile([P, S], f32, tag="qk", name="qk")
                        for h2 in range(2):
                            nc.tensor.matmul(
                                qk[:, 512 * h2:512 * (h2 + 1)],
                                KT[pr][base:base + 64, 128 * j:128 * (j + 1)],
                                QT[pr][base:base + 64, 512 * h2:512 * (h2 + 1)],
                                start=True, stop=True)
                        return qk

                    def emit_epath(hp, j, qk):
                        jl = 128 * j if causal else 0
                        Ej = epool.tile([P, S], bf16, tag="E", name="E")
                        if jl < S:
                            nc.vector.tensor_mul(
                                Ej[:, jl:], eM[hp][:, woff[j]:woff[j] + S - jl],
                                r1bc[hp][:, jl:])
                            nc.vector.tensor_add(qk[:, jl:], qk[:, jl:], Ej[:, jl:])
                        nc.scalar.activation(Ej, qk, Exp)
                        lastE[hp] = Ej

                    def emit_pv(hp, pv, j):
                        for h2 in range(2):
                            nc.tensor.matmul(
                                pv[:, 512 * h2:512 * (h2 + 1)],
                                V2[hp][:, j, :],
                                lastE[hp][:, 512 * h2:512 * (h2 + 1)],
                                start=(j == 0), stop=(j == 7))

                    for j in range(8):
                        qks = {hp: emit_qk(hp, j) for hp, _ in pair}
                        if j > 0:
                            for hp, pv in pair:
                                emit_pv(hp, pv, j - 1)
                        for hp, _ in pair:
                            emit_epath(hp, j, qks[hp])
                        if fillers and j in fillers:
                            fillers[j]()
                    for hp, pv in pair:
                        emit_pv(hp, pv, 7)
                # normalize rows of PV by 1/rowsum2 (ones column) in head
                # PAIRS: Ln,Ln then Exp,Exp keeps the ACT table reloads to 2
                # per pair and leaves the hp -> hp+1 boundary load-free, so
                # the next head's E-exps (and the PE behind them) never stall
                # on a table switch. DVE stages the sum row to partition 0,
                # GPSIMD broadcasts the reciprocal.
                def normalize_pair(hpA, pvA, hpB, pvB):
                    tiles = {}
                    for hp, pv in ((hpA, pvA), (hpB, pvB)):
                        e = hp % 2
                        sum_row = 64 * (1 - e)
                        r2f = miscp.tile([1, S], f32, tag=f"r2f{e}", name=f"r2f{e}")
                        r2s = miscp.tile([1, S], bf16, tag=f"r2s{e}", name=f"r2s{e}")
                        r2b = miscp.tile([P, S], bf16, tag=f"r2b{e}", name=f"r2b{e}")
                        tiles[hp] = (pv, r2f, r2s, r2b)
                        nc.vector.tensor_copy(r2f, pv[sum_row:sum_row + 1, :])
                    for hp in (hpA, hpB):
                        pv, r2f, r2s, r2b = tiles[hp]
                        nc.scalar.activation(r2f, r2f, Ln)
                    for hp in (hpA, hpB):
                        pv, r2f, r2s, r2b = tiles[hp]
                        nc.scalar.activation(r2s, r2f, Exp, scale=-1.0)
                    for hp in (hpA, hpB):
                        pv, r2f, r2s, r2b = tiles[hp]
                        pr, e = hp // 2, hp % 2
                        dlo = 64 * e
                        nc.gpsimd.partition_broadcast(r2b, r2s)
                        nc.vector.tensor_mul(OT[pr][dlo:dlo + 64, :],
                                             pv[dlo:dlo + 64, :],
                                             r2b[dlo:dlo + 64, :])

                for pi in range(2):
                    hpA, hpB = hps[2 * pi], hps[2 * pi + 1]
                    pvA = pvps.tile([P, S], f32, tag="pv", name="pv")
                    pvB = pvps.tile([P, S], f32, tag="pv", name="pv")
                    # round-1 eM prefetch: heads 4/5 thread INTO pair 1's
                    # j-loop (their DMAs are tag-gated on pair 0's last read;
                    # mid-loop ACT slack absorbs the 4us exps instead of
                    # stalling the PE at the pair boundary); heads 6/7 must
                    # wait for pair 1's own tags, so they stay at round end
                    fillers = None
                    if rnd == 0 and pi == 1:
                        fillers = {0: (lambda: em_load(4)), 4: (lambda: em_load(5))}
                    attention_pair(hpA, pvA, hpB, pvB, fillers)
                    normalize_pair(hpA, pvA, hpB, pvB)
                    if rnd == 0 and pi == 1:
                        em_load(6)
                        em_load(7)

        # ---------- phase 3: partial output projection ----------
        with tc.tile_pool(name="os", bufs=3) as osp, \
                tc.tile_pool(name="ops", bufs=2, space="PSUM") as opsum:
            for fo in range(8):
                ot = osp.tile([P, S], bf16, tag="os", name="os")
                for h2 in range(2):
                    ps = opsum.tile([P, 512], f32, tag="op", name="op")
                    for pr in (2, 3):
                        nc.tensor.matmul(ps, wo_sb[:, pr, 128 * fo:128 * (fo + 1)],
                                         OT[pr][:, 512 * h2:512 * (h2 + 1)],
                                         start=(pr == 2), stop=(pr == 3))
                    nc.vector.tensor_add(ot[:, 512 * h2:512 * (h2 + 1)], ps,
                                         oA[fo][:, 512 * h2:512 * (h2 + 1)])
                nc.sync.dma_start(out=out_d[128 * fo:128 * (fo + 1), :], in_=ot)

    nc.compile()
    return nc


def _prep_host(x, str_mat, attn_mask, Wq, bq, Wk, bk, Wv, bv, Wo, bo):
    bf = _np_bf16()
    x = np.asarray(x, np.float32)
    str_mat = np.asarray(str_mat, np.float32)
    attn_mask = np.asarray(attn_mask, np.float32)
    mask = attn_mask[:, 0]  # [b, s, s]
    causal = bool((mask == np.tril(np.ones((S, S), np.float32))[None]).all())
    strT = np.where(mask[:, None] == 0.0, NEG_FILL, str_mat).transpose(0, 1, 3, 2)
    strT = strT.astype(bf)
    xT = x.transpose(0, 2, 1).astype(bf)  # [b, f, s]
    Wq_s = (np.asarray(Wq, np.float32) / D).astype(bf)
    bq_s = (np.asarray(bq, np.float32) / D)
    bias = np.stack([bq_s, np.asarray(bk, np.float32),
                     np.asarray(bv, np.float32)]).astype(bf)
    Wk_c = np.asarray(Wk, np.float32).astype(bf)
    Wv_c = np.asarray(Wv, np.float32).astype(bf)
    Wo_c = np.asarray(Wo, np.float32).astype(bf)
    ident = np.eye(P, dtype=np.float32).astype(bf)
    in_maps = []
    for c in range(NCORES):
        bc, g = c // 2, c % 2
        in_maps.append({
            "xT": np.ascontiguousarray(xT[bc, :, RW * g:RW * (g + 1)]),
            "strT": np.ascontiguousarray(strT[bc, HG * g:HG * (g + 1)]),
            "wq": Wq_s, "wk": Wk_c, "wv": Wv_c,
            "wo": np.ascontiguousarray(Wo_c[RW * g:RW * (g + 1)]),
            "bqkv": bias, "ident": ident,
        })
    return in_maps, causal


def kernel(**inputs):
    from concourse.bass_utils import run_bass_kernel_spmd

    in_maps, causal = _prep_host(**inputs)
    key = ("v2", causal)
    if key not in _CACHE:
        _CACHE[key] = _build_nc(causal=causal)
    nc = _CACHE[key]
    res = run_bass_kernel_spmd(nc, in_maps, core_ids=list(range(NCORES)))
    partials = [np.asarray(r["outT"], np.float32) for r in res.results]
    out = np.stack([partials[2 * bc] + partials[2 * bc + 1] for bc in range(B)])
    out = out.transpose(0, 2, 1) + np.asarray(bo_arr(inputs), np.float32)
    return np.ascontiguousarray(out.astype(np.float32))


def bo_arr(inputs):
    return np.asarray(inputs["bo"], np.float32)
